# revision 13
# baseline (speedup 1.0000x reference)
"""VQ codebook-lookup kernel for Trainium2 (8 NeuronCores, data-parallel over batch).

For each (batch, head, token): find nearest codebook row (L2) among 2048 codes,
output that codebook row. argmin ||q - c||^2 == argmax (q.c - 0.5||c||^2).

Per core (one batch of 8):
  - scores computed on TensorE as 3-term fp16 hi/lo-split matmuls
    (q = qh + ql, c = ch + cl, all four fp16-exact; fp16 products are exact
    in fp32 PSUM, so qh.ch + qh.cl + ql.ch matches full fp32 to ~1e-5 and
    reproduces the reference argmin exactly on this data)
  - fused custom DVE op does bias-add (-0.5||c||^2) + running-max scan +
    argmax-index extraction in ONE 1x pass straight from PSUM
  - GPSIMD indirect DMA gathers the winning codebook rows from DRAM,
    assembled into [128, 1024] out tiles and streamed out during the
    last head's pass
Host side pre-transposes/splits operands (input staging) so no on-chip
transposes are needed.
"""

import numpy as np

import concourse.mybir as mybir
import concourse.tile as tile
from concourse import bacc, bass
from concourse.bass import IndirectOffsetOnAxis
from concourse.bass_utils import run_bass_kernel_spmd

# problem constants (hardcoded per contract)
B = 8  # batch (== n_cores, data-parallel)
N = 2048  # tokens per batch
H = 8  # heads
D = 128  # head dim
M = 2048  # codebook size
NT = N // 128  # 16 n-tiles per head
MB = 4  # m-blocks of 512 per matmul set

f32 = mybir.dt.float32
f32r = mybir.dt.float32r
f16 = mybir.dt.float16
i32 = mybir.dt.int32
bf16 = mybir.dt.bfloat16

# ---------------------------------------------------------------------------
# custom DVE op: one-pass fused (bias-add, running-max scan, argmax index)
# ---------------------------------------------------------------------------
_ARGMAX_OP = None


def _get_argmax_op():
    global _ARGMAX_OP
    if _ARGMAX_OP is not None:
        return _ARGMAX_OP
    import concourse.dve_ops as dve_ops_mod
    from concourse.dve_ops import CUSTOM_DVE_SPECS, OPS, DveOp
    from concourse.dve_spec import (
        AluOp,
        Idx,
        MaxNeg,
        One,
        Spec,
        Src0,
        Src1,
        Zero,
        eq,
        lower,
        maxx,
        scan,
        select,
    )
    from concourse.dve_uop import DveOpSpec

    name = "ARGMAX_BIAS_ANT"
    for existing in OPS:
        if existing.name == name:  # already registered in this process
            _ARGMAX_OP = existing
            return existing

    def _ref(in0, in1, s0, s1, imm2):
        s = in0.astype(np.float32) + in1.astype(np.float32)
        m = np.maximum.accumulate(s, axis=-1)
        idx = np.arange(s.shape[-1], dtype=np.float32)
        fired = np.where(s == m, idx, -1.0).astype(np.float32)
        acc = fired.max(axis=-1).reshape(s.shape[0], 1).astype(np.float32)
        return fired, acc

    s = Src0 + Src1
    m = scan(AluOp.MAX, s)
    body = select(eq(s, m), Idx, Zero - One)
    spec = Spec(body=body, accum=maxx, accum_init=MaxNeg, reference=_ref)
    shas = {}
    for ver in ("v3", "v4"):
        ups = lower(spec, ver=ver)
        shas[ver] = DveOpSpec(name=name, opcode=0, uops=ups, rd1_en=True).sha(ver)
    op = DveOp(name, spec, subdim=False, uops_sha=shas)
    OPS.append(op)
    CUSTOM_DVE_SPECS[name] = spec
    dve_ops_mod._SUB_OPCODE_FOR_NAME[name] = (
        dve_ops_mod._CUSTOM_DVE_ROW_BASE + len(OPS) - 1
    )
    _ARGMAX_OP = op
    return op


# ---------------------------------------------------------------------------
# bass kernel builder
# ---------------------------------------------------------------------------
_NC_CACHE = None


def _build_nc():
    global _NC_CACHE
    if _NC_CACHE is not None:
        return _NC_CACHE
    argmax_op = _get_argmax_op()

    nc = bacc.Bacc("TRN2", target_bir_lowering=False, debug=False, num_devices=B)

    # DRAM I/O (per-core views; each core gets its own batch slice of q)
    d_qh = nc.dram_tensor("qh", [H, D, N], f16, kind="ExternalInput")
    d_ql = nc.dram_tensor("ql", [H, D, N], f16, kind="ExternalInput")
    d_ch = nc.dram_tensor("ch", [H, D, M], f16, kind="ExternalInput")
    d_cl = nc.dram_tensor("cl", [H, D, M], f16, kind="ExternalInput")
    d_c2 = nc.dram_tensor("c2bc", [H, 128, M], f32, kind="ExternalInput")
    d_cb = nc.dram_tensor("cb", [H * M, D], f32, kind="ExternalInput")
    d_out = nc.dram_tensor("out", [N, H * D], f32, kind="ExternalOutput")

    with tile.TileContext(nc) as tc:
        with (
            tc.tile_pool(name="heads", bufs=2) as hp,
            tc.tile_pool(name="outs", bufs=1) as op_pool,
            tc.tile_pool(name="small", bufs=2) as sp,
            tc.tile_pool(name="scr", bufs=1) as scrp,
            tc.tile_pool(name="ps", bufs=2, space="PSUM") as ps,
        ):
            out_tiles = []
            for t in range(NT):
                ot = op_pool.tile([128, H * D], f32, tag=f"out{t}")
                out_tiles.append(ot)
            scratch = scrp.tile([128, M], bf16, tag="scratch")

            for h in range(H):
                s_qh = hp.tile([D, N], f16, tag="qh")
                s_ql = hp.tile([D, N], f16, tag="ql")
                s_ch = hp.tile([D, M], f16, tag="ch")
                s_cl = hp.tile([D, M], f16, tag="cl")
                s_c2 = hp.tile([128, M], f32, tag="c2")
                # order: first matmul needs qh+ch; ql/cl follow; c2 only at argmax
                nc.sync.dma_start(s_qh[:], d_qh[h])
                nc.sync.dma_start(s_ch[:], d_ch[h])
                nc.sync.dma_start(s_cl[:], d_cl[h])
                nc.sync.dma_start(s_ql[:], d_ql[h])
                nc.sync.dma_start(s_c2[:], d_c2[h])

                idx_f = sp.tile([128, NT], f32, tag="idxf")
                idx_i = sp.tile([128, NT], i32, tag="idxi")

                for t in range(NT):
                    psc = ps.tile([128, M], f32, tag="scores")
                    qh_t = s_qh[:, t * 128 : (t + 1) * 128]
                    ql_t = s_ql[:, t * 128 : (t + 1) * 128]
                    # qh.ch blocks (start), qh.cl blocks, ql.ch blocks (stop)
                    for kblk in range(MB):
                        blk = slice(kblk * 512, (kblk + 1) * 512)
                        nc.tensor.matmul(
                            psc[:, blk], qh_t, s_ch[:, blk], start=True, stop=False
                        )
                    for kblk in range(MB):
                        blk = slice(kblk * 512, (kblk + 1) * 512)
                        nc.tensor.matmul(
                            psc[:, blk], qh_t, s_cl[:, blk], start=False, stop=False
                        )
                    for kblk in range(MB):
                        blk = slice(kblk * 512, (kblk + 1) * 512)
                        nc.tensor.matmul(
                            psc[:, blk], ql_t, s_ch[:, blk], start=False, stop=True
                        )
                    # fused bias-add + argmax over m=2048, one DVE pass
                    nc.vector._custom_dve(
                        argmax_op,
                        out=scratch[:],
                        in0=psc[:],
                        in1=s_c2[:],
                        accum_out=idx_f[:, t : t + 1],
                    )
                    # cast f32 index -> i32 on the (idle) scalar engine,
                    # then gather this tile's codebook rows immediately
                    nc.scalar.copy(idx_i[:, t : t + 1], idx_f[:, t : t + 1])
                    nc.gpsimd.indirect_dma_start(
                        out=out_tiles[t][:, h * D : (h + 1) * D],
                        out_offset=None,
                        in_=d_cb[:],
                        in_offset=IndirectOffsetOnAxis(ap=idx_i[:, t : t + 1], axis=0),
                        element_offset=h * M * D,
                    )
                    if h == H - 1:
                        # tile complete after the last head's gather: stream out
                        nc.sync.dma_start(
                            d_out[t * 128 : (t + 1) * 128, :], out_tiles[t][:]
                        )

    nc.compile()
    _NC_CACHE = nc
    return nc


# ---------------------------------------------------------------------------
# host wrapper
# ---------------------------------------------------------------------------


def _round12(x):
    """Truncate fp32 mantissa to 11 explicit bits (exactly representable in
    fp32r's internal 12-bit-rounded format)."""
    return (x.view(np.uint32) & np.uint32(0xFFFFF000)).view(np.float32)


def _prepare_inputs(x, codebooks):
    x = np.ascontiguousarray(np.asarray(x, dtype=np.float32))
    cb = np.ascontiguousarray(np.asarray(codebooks, dtype=np.float32))

    # q transposed per (batch, head): [B, H, D, N]; fp16 hi/lo split
    # (q = qh + ql with both halves fp16-exact -> all products exact in PSUM)
    qT = np.ascontiguousarray(x.reshape(B, N, H, D).transpose(0, 2, 3, 1))
    qh = qT.astype(np.float16)
    ql = (qT - qh.astype(np.float32)).astype(np.float16)

    # codebooks transposed per head: [H, D, M]
    cT = np.ascontiguousarray(cb.transpose(0, 2, 1))
    ch = cT.astype(np.float16)
    cl = (cT - ch.astype(np.float32)).astype(np.float16)

    # -0.5 * ||c||^2 broadcast to 128 partitions: [H, 128, M]
    c2 = -0.5 * (cb.astype(np.float64) ** 2).sum(-1)  # [H, M]
    c2bc = np.ascontiguousarray(
        np.broadcast_to(c2.astype(np.float32)[:, None, :], (H, 128, M))
    )

    cb_flat = np.ascontiguousarray(cb.reshape(H * M, D))

    shared = {
        "ch": np.ascontiguousarray(ch),
        "cl": np.ascontiguousarray(cl),
        "c2bc": c2bc,
        "cb": cb_flat,
    }
    in_maps = []
    for b in range(B):
        m = dict(shared)
        m["qh"] = np.ascontiguousarray(qh[b])
        m["ql"] = np.ascontiguousarray(ql[b])
        in_maps.append(m)
    return in_maps


_LAST_RESULTS = None  # stashed for test harness (exec time inspection)


def kernel(x, codebooks, _trace=False, _trace_kwargs=None):
    global _LAST_RESULTS
    nc = _build_nc()
    in_maps = _prepare_inputs(x, codebooks)
    kw = {}
    if _trace:
        kw["trace"] = True
        kw.update(_trace_kwargs or {})
    res = run_bass_kernel_spmd(nc, in_maps, core_ids=list(range(B)), **kw)
    _LAST_RESULTS = res
    out = np.stack([res.results[b]["out"] for b in range(B)], axis=0)
    return out.astype(np.float32)


# revision 14
# speedup vs baseline: 1.0054x; 1.0054x over previous
"""VQ codebook-lookup kernel for Trainium2 (8 NeuronCores, data-parallel over batch).

For each (batch, head, token): find nearest codebook row (L2) among 2048 codes,
output that codebook row. argmin ||q - c||^2 == argmax (q.c - 0.5||c||^2).

Per core (one batch of 8):
  - scores computed on TensorE as 3-term fp16 hi/lo-split matmuls
    (q = qh + ql, c = ch + cl, all four fp16-exact; fp16 products are exact
    in fp32 PSUM, so qh.ch + qh.cl + ql.ch matches full fp32 to ~1e-5 and
    reproduces the reference argmin exactly on this data)
  - fused custom DVE op does bias-add (-0.5||c||^2) + running-max scan +
    argmax-index extraction in ONE 1x pass straight from PSUM
  - GPSIMD indirect DMA gathers the winning codebook rows from DRAM,
    assembled into [128, 1024] out tiles and streamed out during the
    last head's pass
Host side pre-transposes/splits operands (input staging) so no on-chip
transposes are needed.
"""

import numpy as np

import concourse.mybir as mybir
import concourse.tile as tile
from concourse import bacc, bass
from concourse.bass import IndirectOffsetOnAxis
from concourse.bass_utils import run_bass_kernel_spmd

# problem constants (hardcoded per contract)
B = 8  # batch (== n_cores, data-parallel)
N = 2048  # tokens per batch
H = 8  # heads
D = 128  # head dim
M = 2048  # codebook size
NT = N // 128  # 16 n-tiles per head
MB = 4  # m-blocks of 512 per matmul set

f32 = mybir.dt.float32
f32r = mybir.dt.float32r
f16 = mybir.dt.float16
i32 = mybir.dt.int32
bf16 = mybir.dt.bfloat16

# ---------------------------------------------------------------------------
# custom DVE op: one-pass fused (bias-add, running-max scan, argmax index)
# ---------------------------------------------------------------------------
_ARGMAX_OP = None


def _get_argmax_op():
    global _ARGMAX_OP
    if _ARGMAX_OP is not None:
        return _ARGMAX_OP
    import concourse.dve_ops as dve_ops_mod
    from concourse.dve_ops import CUSTOM_DVE_SPECS, OPS, DveOp
    from concourse.dve_spec import (
        AluOp,
        Idx,
        MaxNeg,
        One,
        Spec,
        Src0,
        Src1,
        Zero,
        eq,
        lower,
        maxx,
        scan,
        select,
    )
    from concourse.dve_uop import DveOpSpec

    name = "ARGMAX_BIAS_ANT"
    for existing in OPS:
        if existing.name == name:  # already registered in this process
            _ARGMAX_OP = existing
            return existing

    def _ref(in0, in1, s0, s1, imm2):
        s = in0.astype(np.float32) + in1.astype(np.float32)
        m = np.maximum.accumulate(s, axis=-1)
        idx = np.arange(s.shape[-1], dtype=np.float32)
        fired = np.where(s == m, idx, -1.0).astype(np.float32)
        acc = fired.max(axis=-1).reshape(s.shape[0], 1).astype(np.float32)
        return fired, acc

    s = Src0 + Src1
    m = scan(AluOp.MAX, s)
    body = select(eq(s, m), Idx, Zero - One)
    spec = Spec(body=body, accum=maxx, accum_init=MaxNeg, reference=_ref)
    shas = {}
    for ver in ("v3", "v4"):
        ups = lower(spec, ver=ver)
        shas[ver] = DveOpSpec(name=name, opcode=0, uops=ups, rd1_en=True).sha(ver)
    op = DveOp(name, spec, subdim=False, uops_sha=shas)
    OPS.append(op)
    CUSTOM_DVE_SPECS[name] = spec
    dve_ops_mod._SUB_OPCODE_FOR_NAME[name] = (
        dve_ops_mod._CUSTOM_DVE_ROW_BASE + len(OPS) - 1
    )
    _ARGMAX_OP = op
    return op


# ---------------------------------------------------------------------------
# bass kernel builder
# ---------------------------------------------------------------------------
_NC_CACHE = None


def _build_nc():
    global _NC_CACHE
    if _NC_CACHE is not None:
        return _NC_CACHE
    argmax_op = _get_argmax_op()

    nc = bacc.Bacc("TRN2", target_bir_lowering=False, debug=False, num_devices=B)

    # DRAM I/O (per-core views; each core gets its own batch slice of q)
    d_qh = nc.dram_tensor("qh", [H, D, N], f16, kind="ExternalInput")
    d_ql = nc.dram_tensor("ql", [H, D, N], f16, kind="ExternalInput")
    d_ch = nc.dram_tensor("ch", [H, D, M], f16, kind="ExternalInput")
    d_cl = nc.dram_tensor("cl", [H, D, M], f16, kind="ExternalInput")
    d_c2 = nc.dram_tensor("c2bc", [H, 128, M], f32, kind="ExternalInput")
    d_cb = nc.dram_tensor("cb", [H * M, D], f32, kind="ExternalInput")
    d_out = nc.dram_tensor("out", [N, H * D], f32, kind="ExternalOutput")

    with tile.TileContext(nc) as tc:
        with (
            tc.tile_pool(name="heads", bufs=2) as hp,
            tc.tile_pool(name="outs", bufs=1) as op_pool,
            tc.tile_pool(name="small", bufs=2) as sp,
            tc.tile_pool(name="scr", bufs=1) as scrp,
            tc.tile_pool(name="ps", bufs=2, space="PSUM") as ps,
        ):
            out_tiles = []
            for t in range(NT):
                ot = op_pool.tile([128, H * D], f32, tag=f"out{t}")
                out_tiles.append(ot)
            scratch = scrp.tile([128, M], bf16, tag="scratch")

            # HAM warm-up: ~5us of dummy matmuls on a zeroed tile fills the
            # preamble->first-data window so real matmuls start at 2.4 GHz
            wz = scrp.tile([128, 512], f16, tag="warmz")
            nc.gpsimd.memset(wz[:], 0)
            psw = ps.tile([128, M], f32, tag="scores")
            for r in range(12):
                nc.tensor.matmul(
                    psw[:, 0:512], wz[:, 0:128], wz[:], start=True, stop=True
                )

            for h in range(H):
                s_qh = hp.tile([D, N], f16, tag="qh")
                s_ql = hp.tile([D, N], f16, tag="ql")
                s_ch = hp.tile([D, M], f16, tag="ch")
                s_cl = hp.tile([D, M], f16, tag="cl")
                s_c2 = hp.tile([128, M], f32, tag="c2")
                # order: first matmul needs qh+ch; ql/cl follow; c2 only at argmax
                nc.sync.dma_start(s_qh[:], d_qh[h])
                nc.sync.dma_start(s_ch[:], d_ch[h])
                nc.sync.dma_start(s_cl[:], d_cl[h])
                nc.sync.dma_start(s_ql[:], d_ql[h])
                nc.sync.dma_start(s_c2[:], d_c2[h])

                idx_f = sp.tile([128, NT], f32, tag="idxf")
                idx_i = sp.tile([128, NT], i32, tag="idxi")

                for t in range(NT):
                    psc = ps.tile([128, M], f32, tag="scores")
                    qh_t = s_qh[:, t * 128 : (t + 1) * 128]
                    ql_t = s_ql[:, t * 128 : (t + 1) * 128]
                    # qh.ch blocks (start), qh.cl blocks, ql.ch blocks (stop)
                    for kblk in range(MB):
                        blk = slice(kblk * 512, (kblk + 1) * 512)
                        nc.tensor.matmul(
                            psc[:, blk], qh_t, s_ch[:, blk], start=True, stop=False
                        )
                    for kblk in range(MB):
                        blk = slice(kblk * 512, (kblk + 1) * 512)
                        nc.tensor.matmul(
                            psc[:, blk], qh_t, s_cl[:, blk], start=False, stop=False
                        )
                    for kblk in range(MB):
                        blk = slice(kblk * 512, (kblk + 1) * 512)
                        nc.tensor.matmul(
                            psc[:, blk], ql_t, s_ch[:, blk], start=False, stop=True
                        )
                    # fused bias-add + argmax over m=2048, one DVE pass
                    nc.vector._custom_dve(
                        argmax_op,
                        out=scratch[:],
                        in0=psc[:],
                        in1=s_c2[:],
                        accum_out=idx_f[:, t : t + 1],
                    )
                    # cast f32 index -> i32 on the (idle) scalar engine,
                    # then gather this tile's codebook rows immediately
                    nc.scalar.copy(idx_i[:, t : t + 1], idx_f[:, t : t + 1])
                    nc.gpsimd.indirect_dma_start(
                        out=out_tiles[t][:, h * D : (h + 1) * D],
                        out_offset=None,
                        in_=d_cb[:],
                        in_offset=IndirectOffsetOnAxis(ap=idx_i[:, t : t + 1], axis=0),
                        element_offset=h * M * D,
                    )
                    if h == H - 1:
                        # tile complete after the last head's gather: stream out
                        nc.sync.dma_start(
                            d_out[t * 128 : (t + 1) * 128, :], out_tiles[t][:]
                        )

    nc.compile()
    _NC_CACHE = nc
    return nc


# ---------------------------------------------------------------------------
# host wrapper
# ---------------------------------------------------------------------------


def _round12(x):
    """Truncate fp32 mantissa to 11 explicit bits (exactly representable in
    fp32r's internal 12-bit-rounded format)."""
    return (x.view(np.uint32) & np.uint32(0xFFFFF000)).view(np.float32)


def _prepare_inputs(x, codebooks):
    x = np.ascontiguousarray(np.asarray(x, dtype=np.float32))
    cb = np.ascontiguousarray(np.asarray(codebooks, dtype=np.float32))

    # q transposed per (batch, head): [B, H, D, N]; fp16 hi/lo split
    # (q = qh + ql with both halves fp16-exact -> all products exact in PSUM)
    qT = np.ascontiguousarray(x.reshape(B, N, H, D).transpose(0, 2, 3, 1))
    qh = qT.astype(np.float16)
    ql = (qT - qh.astype(np.float32)).astype(np.float16)

    # codebooks transposed per head: [H, D, M]
    cT = np.ascontiguousarray(cb.transpose(0, 2, 1))
    ch = cT.astype(np.float16)
    cl = (cT - ch.astype(np.float32)).astype(np.float16)

    # -0.5 * ||c||^2 broadcast to 128 partitions: [H, 128, M]
    c2 = -0.5 * (cb.astype(np.float64) ** 2).sum(-1)  # [H, M]
    c2bc = np.ascontiguousarray(
        np.broadcast_to(c2.astype(np.float32)[:, None, :], (H, 128, M))
    )

    cb_flat = np.ascontiguousarray(cb.reshape(H * M, D))

    shared = {
        "ch": np.ascontiguousarray(ch),
        "cl": np.ascontiguousarray(cl),
        "c2bc": c2bc,
        "cb": cb_flat,
    }
    in_maps = []
    for b in range(B):
        m = dict(shared)
        m["qh"] = np.ascontiguousarray(qh[b])
        m["ql"] = np.ascontiguousarray(ql[b])
        in_maps.append(m)
    return in_maps


_LAST_RESULTS = None  # stashed for test harness (exec time inspection)


def kernel(x, codebooks, _trace=False, _trace_kwargs=None):
    global _LAST_RESULTS
    nc = _build_nc()
    in_maps = _prepare_inputs(x, codebooks)
    kw = {}
    if _trace:
        kw["trace"] = True
        kw.update(_trace_kwargs or {})
    res = run_bass_kernel_spmd(nc, in_maps, core_ids=list(range(B)), **kw)
    _LAST_RESULTS = res
    out = np.stack([res.results[b]["out"] for b in range(B)], axis=0)
    return out.astype(np.float32)
